# revision 1
# baseline (speedup 1.0000x reference)
"""Trainium2 Bass kernel for a dynamic-range compressor (nn_Compressor).

Reference semantics (fp32):
    audio_db = 20*log10(|audio| + 1e-5)
    gr_db    = max((threshold - audio_db) * (1 - 1/ratio), 0)
    scan:    g[t] = g[t-1] + (1-c)*(gr_db[t] - g[t-1]),  c = attack if gr_db[t] > g[t-1] else release
    out      = audio * 10^(-g/20)

Parallelization: the smoother is strongly contracting (error decays by a
factor of max(attack, release) per step), so a chunked scan with a short
warmup prefix converges to fp32-exact. Each of the 8 cores handles a
contiguous 512K-sample shard; inside a core the shard is split into 128
partition-rows of 4096 samples, each with a W-sample warmup halo.

Per-row recurrence, branchless form:
    g[t] = max(attack*g[t-1] + (1-attack)*x[t], release*g[t-1] + (1-release)*x[t])
Solved with the hardware scan instruction (tensor_tensor_scan):
  1. predictor scan: m[t] = max(release*m[t-1], x[t])   (op0=mult, op1=max)
  2. branch mask from sign(x[t] - m[t-1]) -> coeff / (1-coeff) tiles
  3. linear scan:   g[t] = coeff[t]*g[t-1] + (1-coeff[t])*x[t]  (mult, add)
  4. optional second refinement iteration (branch from g instead of m)
One refinement iteration gives ~2e-6 relative output error vs the
sequential reference (validated in numpy); two give ~2e-8.
"""

import math
import os

import numpy as np

import sys

if "/opt/trn_rl_repo" not in sys.path:
    sys.path.insert(0, "/opt/trn_rl_repo")

P = 128            # SBUF partitions
C = 4096           # valid samples per partition-row
W = 32             # warmup samples per row
NCORES = 8
SHARD = P * C      # samples per core
T_TOTAL = NCORES * SHARD
N_ITER = int(os.environ.get("COMP_N_ITER", "1"))
NBLK = int(os.environ.get("COMP_NBLK", "6"))
PAD_VAL = 1e9      # |audio| huge -> gain_reduction = 0 -> matches g=0 initial state


def _register_custom_ops():
    """Fused DVE ops for the branch-select chains, registered at runtime:
      COMP_COEFF_SEL: out = select(in0 > in1, s0, s1)
      COMP_D1_SEL:    out = select(in0 > in1, s0, s1) * in0
    """
    import concourse.dve_ops as dve_ops
    from concourse.dve_spec import Spec, Src0, Src1, C0, C1, select, lower
    from concourse.dve_uop import DveOpSpec

    existing = {o.name: o for o in dve_ops.OPS}
    if "COMP_COEFF_SEL" in existing:
        return existing["COMP_COEFF_SEL"], existing["COMP_D1_SEL"]

    def mk(name, body, reference):
        spec = Spec(body=body, reference=reference)
        row = dve_ops._CUSTOM_DVE_ROW_BASE + len(dve_ops.OPS)
        dve_ops._SUB_OPCODE_FOR_NAME[name] = row
        shas = {}
        for ver in ("v3", "v4"):
            ds = DveOpSpec(name=name, opcode=row, uops=lower(spec, ver=ver),
                           rd1_en=True)
            shas[ver] = ds.sha(ver)
        op = dve_ops.DveOp(name, spec, subdim=False, uops_sha=shas)
        dve_ops.OPS.append(op)
        dve_ops.CUSTOM_DVE_SPECS[name] = spec
        return op

    csel = mk(
        "COMP_COEFF_SEL", select(Src0 > Src1, C0, C1),
        lambda in0, in1, s0, s1, imm2: np.where(in0 > in1, s0, s1).astype(np.float32),
    )
    d1sel = mk(
        "COMP_D1_SEL", select(Src0 > Src1, C0, C1) * Src0,
        lambda in0, in1, s0, s1, imm2: (np.where(in0 > in1, s0, s1) * in0).astype(np.float32),
    )
    return csel, d1sel


def _build_program(thr, ratio, att, rel, n_iter=N_ITER, nblk=NBLK, p=P, c=C, w=W):
    import concourse.bacc as bacc
    import concourse.mybir as mybir
    from concourse.ap import AP
    from concourse.tile import TileContext

    CSEL, D1SEL = _register_custom_ops()

    fp32 = mybir.dt.float32
    AF = mybir.ActivationFunctionType
    ALU = mybir.AluOpType

    shard = p * c
    fd = w + c
    assert fd % nblk == 0, (fd, nblk)
    bw = fd // nblk
    bounds = [i * bw for i in range(nblk + 1)]

    ln10 = math.log(10.0)
    k2 = 1.0 - 1.0 / ratio
    act_scale = -(20.0 * k2) / ln10   # gr_db = relu(act_scale*ln(|a|+eps) + act_bias)
    act_bias = thr * k2
    chalf = (att + rel) / 2.0
    cdiff = (att - rel) / 2.0
    exp_scale = -ln10 / 20.0

    nc = bacc.Bacc("TRN2", target_bir_lowering=False)

    def reg_const(val):
        val = float(val)
        key = (fp32, val)
        if key not in nc.const_aps.aps:
            t = nc.alloc_sbuf_tensor(f"const-f32-{val}", [128, 1], fp32)
            nc.gpsimd.memset(t.ap(), val)
            nc.const_aps.aps[key] = t.ap()

    reg_const(1e-5)
    reg_const(act_bias)
    nc.all_engine_barrier()

    ain = nc.dram_tensor("a_in", [shard + w], fp32, kind="ExternalInput")
    aout = nc.dram_tensor("a_out", [shard], fp32, kind="ExternalOutput")
    ain_h = ain.ap().tensor
    aout_h = aout.ap().tensor

    with TileContext(nc) as tc:
        with tc.tile_pool(name="pool", bufs=1) as pool:
            aud = pool.tile([p, fd], fp32, tag="aud")
            tA = pool.tile([p, fd], fp32, tag="tA")     # abs scratch, later sign / gain
            tB = pool.tile([p, fd], fp32, tag="tB")     # ln scratch, later s / out
            x = pool.tile([p, fd], fp32, tag="x")       # gain_reduction_db
            relt = pool.tile([p, fd], fp32, tag="relt")  # const release tile
            m = pool.tile([p, fd], fp32, tag="m")       # predictor scan
            coeff = pool.tile([p, fd], fp32, tag="coeff")
            d1 = pool.tile([p, fd], fp32, tag="d1")
            g = pool.tile([p, fd], fp32, tag="g")
            if n_iter > 1:
                g2 = pool.tile([p, fd], fp32, tag="g2")
            else:
                g2 = None
            scratch = pool.tile([p, 1], fp32, tag="scratch")

            # The hardware scan instruction (S2S2D2_STT encoding) has room
            # for very few semaphore waits; a 1-column vector copy "touches"
            # the scalar-engine output x right before each predictor scan so
            # the cross-engine wait lands on the copy instead.
            nc.gpsimd.memset(relt[:], rel)

            for b in range(nblk):
                c0, c1 = bounds[b], bounds[b + 1]
                blk = slice(c0, c1)
                # rows: aud[pp, col] = ain[pp*c + col]; rows overlap by w
                src = AP(ain_h, c0, [[c, p], [1, c1 - c0]])
                nc.sync.dma_start(out=aud[:, blk], in_=src)

                # front-end elementwise (scalar engine)
                nc.scalar.activation(tA[:, blk], aud[:, blk], AF.Abs)
                nc.scalar.activation(tB[:, blk], tA[:, blk], AF.Ln, bias=1e-5)
                nc.scalar.activation(x[:, blk], tB[:, blk], AF.Relu,
                                     bias=act_bias, scale=act_scale)

                # predictor scan: m[t] = max(rel*m[t-1], x[t])
                nc.vector.tensor_copy(scratch[:, 0:1], x[:, c1 - 1:c1])
                nc.vector.tensor_tensor_scan(
                    m[:, blk], relt[:, blk], x[:, blk],
                    initial=0.0 if b == 0 else m[:, c0 - 1:c0],
                    op0=ALU.mult, op1=ALU.max)

                prev = m
                cur = g
                for it in range(n_iter):
                    # coeff = select(x[t] > prev[t-1], att, rel)
                    # d1    = select(x[t] > prev[t-1], 1-att, 1-rel) * x[t]
                    if b == 0:
                        nc.vector.memset(coeff[:, 0:1], rel)
                        nc.vector.tensor_scalar_mul(d1[:, 0:1], x[:, 0:1], 1.0 - rel)
                        nc.vector._custom_dve(
                            CSEL, out=coeff[:, 1:c1], in0=x[:, 1:c1],
                            in1=prev[:, 0:c1 - 1], s0=att, s1=rel)
                        nc.vector._custom_dve(
                            D1SEL, out=d1[:, 1:c1], in0=x[:, 1:c1],
                            in1=prev[:, 0:c1 - 1], s0=1.0 - att, s1=1.0 - rel)
                    else:
                        nc.vector._custom_dve(
                            CSEL, out=coeff[:, blk], in0=x[:, blk],
                            in1=prev[:, c0 - 1:c1 - 1], s0=att, s1=rel)
                        nc.vector._custom_dve(
                            D1SEL, out=d1[:, blk], in0=x[:, blk],
                            in1=prev[:, c0 - 1:c1 - 1], s0=1.0 - att, s1=1.0 - rel)
                    nc.vector.tensor_tensor_scan(
                        cur[:, blk], coeff[:, blk], d1[:, blk],
                        initial=0.0 if b == 0 else cur[:, c0 - 1:c0],
                        op0=ALU.mult, op1=ALU.add)
                    prev, cur = cur, (g2 if prev is m else prev)

                gfin = prev
                # back-end: gain = exp(-ln10/20 * g); out = audio*gain
                v0 = max(c0, w)
                nc.scalar.activation(tA[:, v0:c1], gfin[:, v0:c1], AF.Exp,
                                     scale=exp_scale)
                nc.vector.tensor_tensor(
                    tB[:, v0:c1], aud[:, v0:c1], tA[:, v0:c1], op=ALU.mult)
                dst = AP(aout_h, v0 - w, [[c, p], [1, c1 - v0]])
                nc.sync.dma_start(out=dst, in_=tB[:, v0:c1])

    if not nc.is_finalized():
        nc.finalize()
    return nc


_CACHE = {}


def _get_program(thr, ratio, att, rel):
    key = (float(thr), float(ratio), float(att), float(rel), N_ITER, NBLK)
    if key not in _CACHE:
        _CACHE[key] = _build_program(*key[:4], n_iter=N_ITER, nblk=NBLK)
    return _CACHE[key]


def kernel(audio, threshold, ratio, attack, release):
    from concourse.bass_utils import run_bass_kernel_spmd

    audio = np.asarray(audio, dtype=np.float32)
    assert audio.shape == (T_TOTAL,), audio.shape
    thr = float(np.asarray(threshold))
    rat = float(np.asarray(ratio))
    att = float(np.asarray(attack))
    rel = float(np.asarray(release))

    nc = _get_program(thr, rat, att, rel)

    padded = np.concatenate([np.full(W, PAD_VAL, dtype=np.float32), audio])
    in_maps = [
        {"a_in": padded[cid * SHARD: cid * SHARD + SHARD + W]}
        for cid in range(NCORES)
    ]
    res = run_bass_kernel_spmd(nc, in_maps, list(range(NCORES)))
    out = np.concatenate([res.results[cid]["a_out"] for cid in range(NCORES)])
    return out.astype(np.float32)



# revision 2
# speedup vs baseline: 1.2284x; 1.2284x over previous
"""Trainium2 Bass kernel for a dynamic-range compressor (nn_Compressor).

Reference semantics (fp32):
    audio_db = 20*log10(|audio| + 1e-5)
    gr_db    = max((threshold - audio_db) * (1 - 1/ratio), 0)
    scan:    g[t] = g[t-1] + (1-c)*(gr_db[t] - g[t-1]),  c = attack if gr_db[t] > g[t-1] else release
    out      = audio * 10^(-g/20)

Key observations exploited here:
  * The smoother is strongly contracting (c <= 0.1), so g[t-1] ~= x[t-1]:
    the branch can be predicted from x[t] vs x[t-1] (a pure shifted view)
    instead of a second hardware scan.  Validated: 5e-5 relative output
    error vs the sequential fp32 reference (gate is 2e-2).
  * Work in the ln(a^2) domain: sq = a^2 (Square), L = ln(sq + 1e-10).
    Then x = (gamma/2) * relu(K2 - L) with K2 = 2*ln(10^(thr/20)), and the
    comparison x[t] > x[t-1] becomes (L[t] < L[t-1]) & (L[t] < K2), which a
    single fused DVE op evaluates, folding away the Abs and Relu scalar
    passes. The leftover gamma/2 scale folds into the final Exp's scale.
  * Scan: g[t] = coeff[t]*g[t-1] + d1[t] via the hardware scan instruction
    (tensor_tensor_scan, op0=mult, op1=add).

Per core: 512K-sample contiguous shard as 128 partition-rows of 4096
samples with a W-sample warmup halo per row.

Engine split per block: scalar: Square, Ln, Exp;  vector: CSEL3, D1SEL3
(fused select ops), scan;  gpsimd: final out = audio * gain;  sync: DMA.
All scalar activations live in one table set (natural_log_exp_and_others);
the per-function set choice is pinned so only one ACT_TABLE_LOAD happens,
and a dummy Exp at program start overlaps that load with the first DMA.
"""

import math
import os

import numpy as np

import sys

if "/opt/trn_rl_repo" not in sys.path:
    sys.path.insert(0, "/opt/trn_rl_repo")

P = 128            # SBUF partitions
C = 4096           # valid samples per partition-row
W = 32             # warmup samples per row
NCORES = 8
SHARD = P * C      # samples per core
T_TOTAL = NCORES * SHARD
NBLK = int(os.environ.get("COMP_NBLK", "6"))
MULT_ENGINE = os.environ.get("COMP_MULT", "gpsimd")  # gpsimd | vector
PAD_VAL = 1e9      # |audio| huge -> gain_reduction = 0 -> matches g=0 initial state


def _register_custom_ops():
    """Fused DVE ops, registered at runtime:
      COMP_CSEL3: out = select((in0 < in1) & (in0 < K2), s0, s1)
      COMP_D1SEL3: out = select((in0 < in1) & (in0 < K2), s0, s1) * max(K2 - in0, 0)
    (in0 = L[t], in1 = L[t-1], K2 via imm2.)
    """
    import concourse.dve_ops as dve_ops
    from concourse.dve_spec import Spec, Src0, Src1, C0, C1, C2, Zero, select, maxx, lower
    from concourse.dve_uop import DveOpSpec

    existing = {o.name: o for o in dve_ops.OPS}
    if "COMP_CSEL3" in existing:
        return existing["COMP_CSEL3"], existing["COMP_D1SEL3"]

    def mk(name, body, reference):
        spec = Spec(body=body, reference=reference)
        row = dve_ops._CUSTOM_DVE_ROW_BASE + len(dve_ops.OPS)
        dve_ops._SUB_OPCODE_FOR_NAME[name] = row
        shas = {}
        for ver in ("v3", "v4"):
            ds = DveOpSpec(name=name, opcode=row, uops=lower(spec, ver=ver),
                           rd1_en=True)
            shas[ver] = ds.sha(ver)
        op = dve_ops.DveOp(name, spec, subdim=False, uops_sha=shas)
        dve_ops.OPS.append(op)
        dve_ops.CUSTOM_DVE_SPECS[name] = spec
        return op

    csel = mk(
        "COMP_CSEL3",
        select((Src0 < Src1) & (Src0 < C2), C0, C1),
        lambda in0, in1, s0, s1, imm2: np.where(
            (in0 < in1) & (in0 < imm2), s0, s1).astype(np.float32),
    )
    d1sel = mk(
        "COMP_D1SEL3",
        select((Src0 < Src1) & (Src0 < C2), C0, C1) * maxx(C2 - Src0, Zero),
        lambda in0, in1, s0, s1, imm2: (np.where(
            (in0 < in1) & (in0 < imm2), s0, s1)
            * np.maximum(imm2 - in0, 0.0)).astype(np.float32),
    )
    return csel, d1sel


def _pin_act_table_set(nc):
    """All four activation functions used here (Square, Ln, Exp) live in the
    `natural_log_exp_and_others` set; the compiler's per-function set choice
    would otherwise thrash between sets (one ACT_TABLE_LOAD is ~2.7us).
    Drop these functions from every other set in the (cached) table map so
    the insertion pass resolves them all to the one set that really does
    contain them."""
    import concourse.mybir as mybir
    from concourse.hw_specs import get_activation_tables

    AF = mybir.ActivationFunctionType
    tabs = get_activation_tables(nc.m.arch)
    mine = {AF.Square, AF.Ln, AF.Exp}
    keep = "natural_log_exp_and_others"
    assert keep in tabs and mine <= tabs[keep]
    for name, fns in tabs.items():
        if name != keep:
            fns -= mine


def _build_program(thr, ratio, att, rel, nblk=NBLK, p=P, c=C, w=W):
    import concourse.bacc as bacc
    import concourse.mybir as mybir
    from concourse.ap import AP
    from concourse.tile import TileContext

    CSEL3, D1SEL3 = _register_custom_ops()

    fp32 = mybir.dt.float32
    AF = mybir.ActivationFunctionType
    ALU = mybir.AluOpType

    shard = p * c
    fd = w + c
    assert fd % nblk == 0, (fd, nblk)
    bw = fd // nblk
    bounds = [i * bw for i in range(nblk + 1)]

    k2 = 1.0 - 1.0 / ratio
    K2 = 2.0 * (thr / 20.0) * math.log(10.0)   # = 2*ln(10^(thr/20))
    exp_scale = -k2 / 2.0

    nc = bacc.Bacc("TRN2", target_bir_lowering=False)
    _pin_act_table_set(nc)

    def reg_const(val):
        val = float(val)
        key = (fp32, val)
        if key not in nc.const_aps.aps:
            t = nc.alloc_sbuf_tensor(f"const-f32-{val}", [128, 1], fp32)
            nc.gpsimd.memset(t.ap(), val)
            nc.const_aps.aps[key] = t.ap()

    reg_const(1e-10)
    nc.all_engine_barrier()

    ain = nc.dram_tensor("a_in", [shard + w], fp32, kind="ExternalInput")
    aout = nc.dram_tensor("a_out", [shard], fp32, kind="ExternalOutput")
    ain_h = ain.ap().tensor
    aout_h = aout.ap().tensor

    with TileContext(nc) as tc:
        with tc.tile_pool(name="pool", bufs=1) as pool:
            aud = pool.tile([p, fd], fp32, tag="aud")
            sq = pool.tile([p, fd], fp32, tag="sq")
            L = pool.tile([p, fd], fp32, tag="L")
            coeff = pool.tile([p, fd], fp32, tag="coeff")
            d1 = pool.tile([p, fd], fp32, tag="d1")
            h = pool.tile([p, fd], fp32, tag="h")
            gain = pool.tile([p, fd], fp32, tag="gain")
            outb = pool.tile([p, fd], fp32, tag="outb")
            scratch = pool.tile([p, 1], fp32, tag="scratch")

            # Dummy activation: hoists the single ACT_TABLE_LOAD to t=0 so
            # it overlaps the first DMA instead of serializing before the
            # first Square.
            zero_ap = nc.const_aps.tensor(0.0, (p, 1))
            nc.scalar.activation(scratch[:, 0:1], zero_ap, AF.Exp)

            for b in range(nblk):
                c0, c1 = bounds[b], bounds[b + 1]
                blk = slice(c0, c1)
                # rows: aud[pp, col] = ain[pp*c + col]; rows overlap by w
                src = AP(ain_h, c0, [[c, p], [1, c1 - c0]])
                nc.sync.dma_start(out=aud[:, blk], in_=src)

                # front-end (scalar): sq = a^2 ; L = ln(sq + 1e-10)
                nc.scalar.activation(sq[:, blk], aud[:, blk], AF.Square)
                nc.scalar.activation(L[:, blk], sq[:, blk], AF.Ln, bias=1e-10)

                # coeff/d1 from the shifted-L branch predictor
                if b == 0:
                    # col 0: in0 == in1 -> cond false -> release branch
                    nc.vector._custom_dve(
                        CSEL3, out=coeff[:, 0:1], in0=L[:, 0:1],
                        in1=L[:, 0:1], s0=att, s1=rel, imm2=K2)
                    nc.vector._custom_dve(
                        D1SEL3, out=d1[:, 0:1], in0=L[:, 0:1],
                        in1=L[:, 0:1], s0=1.0 - att, s1=1.0 - rel, imm2=K2)
                    nc.vector._custom_dve(
                        CSEL3, out=coeff[:, 1:c1], in0=L[:, 1:c1],
                        in1=L[:, 0:c1 - 1], s0=att, s1=rel, imm2=K2)
                    nc.vector._custom_dve(
                        D1SEL3, out=d1[:, 1:c1], in0=L[:, 1:c1],
                        in1=L[:, 0:c1 - 1], s0=1.0 - att, s1=1.0 - rel,
                        imm2=K2)
                else:
                    nc.vector._custom_dve(
                        CSEL3, out=coeff[:, blk], in0=L[:, blk],
                        in1=L[:, c0 - 1:c1 - 1], s0=att, s1=rel, imm2=K2)
                    nc.vector._custom_dve(
                        D1SEL3, out=d1[:, blk], in0=L[:, blk],
                        in1=L[:, c0 - 1:c1 - 1], s0=1.0 - att, s1=1.0 - rel,
                        imm2=K2)

                # scan: h[t] = coeff[t]*h[t-1] + d1[t]
                nc.vector.tensor_tensor_scan(
                    h[:, blk], coeff[:, blk], d1[:, blk],
                    initial=0.0 if b == 0 else h[:, c0 - 1:c0],
                    op0=ALU.mult, op1=ALU.add)

                # back-end: gain = exp(-k2/2 * h); out = audio*gain
                v0 = max(c0, w)
                nc.scalar.activation(gain[:, v0:c1], h[:, v0:c1], AF.Exp,
                                     scale=exp_scale)
                if MULT_ENGINE == "gpsimd":
                    nc.gpsimd.tensor_tensor(
                        out=outb[:, v0:c1], in0=aud[:, v0:c1],
                        in1=gain[:, v0:c1], op=ALU.mult)
                else:
                    nc.vector.tensor_tensor(
                        outb[:, v0:c1], aud[:, v0:c1], gain[:, v0:c1],
                        op=ALU.mult)
                dst = AP(aout_h, v0 - w, [[c, p], [1, c1 - v0]])
                nc.sync.dma_start(out=dst, in_=outb[:, v0:c1])

    if not nc.is_finalized():
        nc.finalize()
    return nc


_CACHE = {}


def _get_program(thr, ratio, att, rel):
    key = (float(thr), float(ratio), float(att), float(rel), NBLK, MULT_ENGINE)
    if key not in _CACHE:
        _CACHE[key] = _build_program(*key[:4], nblk=NBLK)
    return _CACHE[key]


def kernel(audio, threshold, ratio, attack, release):
    from concourse.bass_utils import run_bass_kernel_spmd

    audio = np.asarray(audio, dtype=np.float32)
    assert audio.shape == (T_TOTAL,), audio.shape
    thr = float(np.asarray(threshold))
    rat = float(np.asarray(ratio))
    att = float(np.asarray(attack))
    rel = float(np.asarray(release))

    nc = _get_program(thr, rat, att, rel)

    padded = np.concatenate([np.full(W, PAD_VAL, dtype=np.float32), audio])
    in_maps = [
        {"a_in": padded[cid * SHARD: cid * SHARD + SHARD + W]}
        for cid in range(NCORES)
    ]
    res = run_bass_kernel_spmd(nc, in_maps, list(range(NCORES)))
    out = np.concatenate([res.results[cid]["a_out"] for cid in range(NCORES)])
    return out.astype(np.float32)


# revision 6
# speedup vs baseline: 1.2321x; 1.0030x over previous
"""Trainium2 Bass kernel for a dynamic-range compressor (nn_Compressor).

Reference semantics (fp32):
    audio_db = 20*log10(|audio| + 1e-5)
    gr_db    = max((threshold - audio_db) * (1 - 1/ratio), 0)
    scan:    g[t] = g[t-1] + (1-c)*(gr_db[t] - g[t-1]),  c = attack if gr_db[t] > g[t-1] else release
    out      = audio * 10^(-g/20)

Key observations exploited here:
  * The smoother is strongly contracting (c <= 0.1), so g[t-1] ~= x[t-1]:
    the branch can be predicted from x[t] vs x[t-1] (a pure shifted view)
    instead of a second hardware scan.  Validated: 5e-5 relative output
    error vs the sequential fp32 reference (gate is 2e-2).
  * Work in the ln(a^2) domain: sq = a^2 (Square), L = ln(sq + 1e-10).
    Then x = (gamma/2) * relu(K2 - L) with K2 = 2*ln(10^(thr/20)), and the
    comparison x[t] > x[t-1] becomes (L[t] < L[t-1]) & (L[t] < K2), which a
    single fused DVE op evaluates, folding away the Abs and Relu scalar
    passes. The leftover gamma/2 scale folds into the final Exp's scale.
  * Scan: g[t] = coeff[t]*g[t-1] + d1[t] via the hardware scan instruction
    (tensor_tensor_scan, op0=mult, op1=add).

Per core: 512K-sample contiguous shard as 128 partition-rows of 4096
samples with a W-sample warmup halo per row.

Engine split per block: scalar: Square, Ln, Exp;  vector: CSEL3, D1SEL3
(fused select ops), scan;  gpsimd: final out = audio * gain;  sync: DMA.
All scalar activations live in one table set (natural_log_exp_and_others);
the per-function set choice is pinned so only one ACT_TABLE_LOAD happens,
and a dummy Exp at program start overlaps that load with the first DMA.
"""

import math
import os

import numpy as np

import sys

if "/opt/trn_rl_repo" not in sys.path:
    sys.path.insert(0, "/opt/trn_rl_repo")

P = 128            # SBUF partitions
C = 4096           # valid samples per partition-row
W = 32             # warmup samples per row
NCORES = 8
SHARD = P * C      # samples per core
T_TOTAL = NCORES * SHARD
NBLK = int(os.environ.get("COMP_NBLK", "6"))
MULT_ENGINE = os.environ.get("COMP_MULT", "vector")  # gpsimd | vector
# Column bounds within a row (fd = W + C). A small first block shortens the
# pipeline fill (first DMA + Square/Ln before the vector engine can start);
# a smaller last block shortens the drain (exp + mult + DMA-out after the
# last scan). Override with COMP_BOUNDS="128,960,..." (must sum to fd).
_BOUNDS_ENV = os.environ.get("COMP_BOUNDS", "")
PAD_VAL = 1e9      # |audio| huge -> gain_reduction = 0 -> matches g=0 initial state


def _register_custom_ops():
    """Fused DVE ops, registered at runtime:
      COMP_CSEL3: out = select((in0 < in1) & (in0 < K2), s0, s1)
      COMP_D1SEL3: out = select((in0 < in1) & (in0 < K2), s0, s1) * max(K2 - in0, 0)
    (in0 = L[t], in1 = L[t-1], K2 via imm2.)
    """
    import concourse.dve_ops as dve_ops
    from concourse.dve_spec import Spec, Src0, Src1, C0, C1, C2, Zero, select, maxx, lower
    from concourse.dve_uop import DveOpSpec

    existing = {o.name: o for o in dve_ops.OPS}
    if "COMP_CSEL3" in existing:
        return existing["COMP_CSEL3"], existing["COMP_D1SEL3"]

    def mk(name, body, reference):
        spec = Spec(body=body, reference=reference)
        row = dve_ops._CUSTOM_DVE_ROW_BASE + len(dve_ops.OPS)
        dve_ops._SUB_OPCODE_FOR_NAME[name] = row
        shas = {}
        for ver in ("v3", "v4"):
            ds = DveOpSpec(name=name, opcode=row, uops=lower(spec, ver=ver),
                           rd1_en=True)
            shas[ver] = ds.sha(ver)
        op = dve_ops.DveOp(name, spec, subdim=False, uops_sha=shas)
        dve_ops.OPS.append(op)
        dve_ops.CUSTOM_DVE_SPECS[name] = spec
        return op

    csel = mk(
        "COMP_CSEL3",
        select((Src0 < Src1) & (Src0 < C2), C0, C1),
        lambda in0, in1, s0, s1, imm2: np.where(
            (in0 < in1) & (in0 < imm2), s0, s1).astype(np.float32),
    )
    d1sel = mk(
        "COMP_D1SEL3",
        select((Src0 < Src1) & (Src0 < C2), C0, C1) * maxx(C2 - Src0, Zero),
        lambda in0, in1, s0, s1, imm2: (np.where(
            (in0 < in1) & (in0 < imm2), s0, s1)
            * np.maximum(imm2 - in0, 0.0)).astype(np.float32),
    )
    return csel, d1sel


def _pin_act_table_set(nc):
    """All four activation functions used here (Square, Ln, Exp) live in the
    `natural_log_exp_and_others` set; the compiler's per-function set choice
    would otherwise thrash between sets (one ACT_TABLE_LOAD is ~2.7us).
    Drop these functions from every other set in the (cached) table map so
    the insertion pass resolves them all to the one set that really does
    contain them."""
    import concourse.mybir as mybir
    from concourse.hw_specs import get_activation_tables

    AF = mybir.ActivationFunctionType
    tabs = get_activation_tables(nc.m.arch)
    mine = {AF.Square, AF.Ln, AF.Exp}
    keep = "natural_log_exp_and_others"
    assert keep in tabs and mine <= tabs[keep]
    for name, fns in tabs.items():
        if name != keep:
            fns -= mine


def _build_program(thr, ratio, att, rel, nblk=NBLK, p=P, c=C, w=W):
    import concourse.bacc as bacc
    import concourse.mybir as mybir
    from concourse.ap import AP
    from concourse.tile import TileContext

    CSEL3, D1SEL3 = _register_custom_ops()

    fp32 = mybir.dt.float32
    AF = mybir.ActivationFunctionType
    ALU = mybir.AluOpType

    shard = p * c
    fd = w + c
    if _BOUNDS_ENV:
        widths = [int(x) for x in _BOUNDS_ENV.split(",")]
    else:
        # taper: small first block (fast fill), small last block (fast drain)
        body = nblk - 2
        first = 128
        last = 544
        bw = (fd - first - last) // body
        widths = [first] + [bw] * body + [last]
        widths[-2] += fd - sum(widths)
    assert sum(widths) == fd and all(x > 0 for x in widths), widths
    nblk = len(widths)
    bounds = [0]
    for x in widths:
        bounds.append(bounds[-1] + x)

    k2 = 1.0 - 1.0 / ratio
    K2 = 2.0 * (thr / 20.0) * math.log(10.0)   # = 2*ln(10^(thr/20))
    exp_scale = -k2 / 2.0

    nc = bacc.Bacc("TRN2", target_bir_lowering=False)
    _pin_act_table_set(nc)

    ain = nc.dram_tensor("a_in", [shard + w], fp32, kind="ExternalInput")
    aout = nc.dram_tensor("a_out", [shard], fp32, kind="ExternalOutput")
    ain_h = ain.ap().tensor
    aout_h = aout.ap().tensor

    with TileContext(nc) as tc:
        with tc.tile_pool(name="pool", bufs=1) as pool:
            aud = pool.tile([p, fd], fp32, tag="aud")
            sq = pool.tile([p, fd], fp32, tag="sq")
            L = pool.tile([p, fd], fp32, tag="L")
            coeff = pool.tile([p, fd], fp32, tag="coeff")
            d1 = pool.tile([p, fd], fp32, tag="d1")
            h = pool.tile([p, fd], fp32, tag="h")
            gain = pool.tile([p, fd], fp32, tag="gain")
            outb = pool.tile([p, fd], fp32, tag="outb")
            scratch = pool.tile([p, 1], fp32, tag="scratch")
            eps = pool.tile([p, 1], fp32, tag="eps")

            # Ln bias constant, Tile-tracked (avoids an all-engine barrier).
            nc.gpsimd.memset(eps[:, 0:1], 1e-10)
            # Dummy activation: hoists the single ACT_TABLE_LOAD to t=0 so
            # it overlaps the first DMA instead of serializing before the
            # first Square.
            zero_ap = nc.const_aps.tensor(0.0, (p, 1))
            nc.scalar.activation(scratch[:, 0:1], zero_ap, AF.Exp)

            def back_end(b):
                # gain = exp(-k2/2 * h); out = audio*gain
                c0, c1 = bounds[b], bounds[b + 1]
                v0 = max(c0, w)
                nc.scalar.activation(gain[:, v0:c1], h[:, v0:c1], AF.Exp,
                                     scale=exp_scale)
                if MULT_ENGINE == "gpsimd":
                    nc.gpsimd.tensor_tensor(
                        out=outb[:, v0:c1], in0=aud[:, v0:c1],
                        in1=gain[:, v0:c1], op=ALU.mult)
                else:
                    nc.vector.tensor_tensor(
                        outb[:, v0:c1], aud[:, v0:c1], gain[:, v0:c1],
                        op=ALU.mult)
                dst = AP(aout_h, v0 - w, [[c, p], [1, c1 - v0]])
                nc.sync.dma_start(out=dst, in_=outb[:, v0:c1])

            for b in range(nblk):
                c0, c1 = bounds[b], bounds[b + 1]
                blk = slice(c0, c1)
                # rows: aud[pp, col] = ain[pp*c + col]; rows overlap by w
                src = AP(ain_h, c0, [[c, p], [1, c1 - c0]])
                nc.sync.dma_start(out=aud[:, blk], in_=src)

                # front-end (scalar): sq = a^2 ; L = ln(sq + 1e-10)
                nc.scalar.activation(sq[:, blk], aud[:, blk], AF.Square)
                nc.scalar.activation(L[:, blk], sq[:, blk], AF.Ln,
                                     bias=eps[:, 0:1])

                # coeff/d1 from the shifted-L branch predictor
                if b == 0:
                    # col 0: in0 == in1 -> cond false -> release branch
                    nc.vector._custom_dve(
                        CSEL3, out=coeff[:, 0:1], in0=L[:, 0:1],
                        in1=L[:, 0:1], s0=att, s1=rel, imm2=K2)
                    nc.vector._custom_dve(
                        D1SEL3, out=d1[:, 0:1], in0=L[:, 0:1],
                        in1=L[:, 0:1], s0=1.0 - att, s1=1.0 - rel, imm2=K2)
                    nc.vector._custom_dve(
                        CSEL3, out=coeff[:, 1:c1], in0=L[:, 1:c1],
                        in1=L[:, 0:c1 - 1], s0=att, s1=rel, imm2=K2)
                    nc.vector._custom_dve(
                        D1SEL3, out=d1[:, 1:c1], in0=L[:, 1:c1],
                        in1=L[:, 0:c1 - 1], s0=1.0 - att, s1=1.0 - rel,
                        imm2=K2)
                else:
                    nc.vector._custom_dve(
                        CSEL3, out=coeff[:, blk], in0=L[:, blk],
                        in1=L[:, c0 - 1:c1 - 1], s0=att, s1=rel, imm2=K2)
                    nc.vector._custom_dve(
                        D1SEL3, out=d1[:, blk], in0=L[:, blk],
                        in1=L[:, c0 - 1:c1 - 1], s0=1.0 - att, s1=1.0 - rel,
                        imm2=K2)

                # scan: h[t] = coeff[t]*h[t-1] + d1[t]
                nc.vector.tensor_tensor_scan(
                    h[:, blk], coeff[:, blk], d1[:, blk],
                    initial=0.0 if b == 0 else h[:, c0 - 1:c0],
                    op0=ALU.mult, op1=ALU.add)

                # back-end of the previous block: its gain/mult/DMA-out run
                # while this block's scan is on the vector queue, and the
                # emission order keeps the next block's front-end ahead of
                # the previous block's exp on the in-order scalar queue.
                if b > 0:
                    back_end(b - 1)
            back_end(nblk - 1)

    if not nc.is_finalized():
        nc.finalize()
    return nc


_CACHE = {}


def _get_program(thr, ratio, att, rel):
    key = (float(thr), float(ratio), float(att), float(rel), NBLK, MULT_ENGINE)
    if key not in _CACHE:
        _CACHE[key] = _build_program(*key[:4], nblk=NBLK)
    return _CACHE[key]


def kernel(audio, threshold, ratio, attack, release):
    from concourse.bass_utils import run_bass_kernel_spmd

    audio = np.asarray(audio, dtype=np.float32)
    assert audio.shape == (T_TOTAL,), audio.shape
    thr = float(np.asarray(threshold))
    rat = float(np.asarray(ratio))
    att = float(np.asarray(attack))
    rel = float(np.asarray(release))

    nc = _get_program(thr, rat, att, rel)

    padded = np.concatenate([np.full(W, PAD_VAL, dtype=np.float32), audio])
    in_maps = [
        {"a_in": padded[cid * SHARD: cid * SHARD + SHARD + W]}
        for cid in range(NCORES)
    ]
    res = run_bass_kernel_spmd(nc, in_maps, list(range(NCORES)))
    out = np.concatenate([res.results[cid]["a_out"] for cid in range(NCORES)])
    return out.astype(np.float32)


# revision 9
# speedup vs baseline: 1.3189x; 1.0705x over previous
"""Trainium2 Bass kernel for a dynamic-range compressor (nn_Compressor).

Reference semantics (fp32):
    audio_db = 20*log10(|audio| + 1e-5)
    gr_db    = max((threshold - audio_db) * (1 - 1/ratio), 0)
    scan:    g[t] = g[t-1] + (1-c)*(gr_db[t] - g[t-1]),  c = attack if gr_db[t] > g[t-1] else release
    out      = audio * 10^(-g/20)

Key observations exploited here:
  * The smoother is strongly contracting (c <= 0.1), so g[t-1] ~= x[t-1]:
    the branch can be predicted from x[t] vs x[t-1] (a pure shifted view)
    instead of a second hardware scan.  Validated: 5e-5 relative output
    error vs the sequential fp32 reference (gate is 2e-2).
  * Work in the ln(a^2) domain: sq = a^2 (Square), L = ln(sq + 1e-10).
    Then x = (gamma/2) * relu(K2 - L) with K2 = 2*ln(10^(thr/20)), and the
    comparison x[t] > x[t-1] becomes (L[t] < L[t-1]) & (L[t] < K2), which a
    single fused DVE op evaluates, folding away the Abs and Relu scalar
    passes. The leftover gamma/2 scale folds into the final Exp's scale.
  * Scan: g[t] = coeff[t]*g[t-1] + d1[t] via the hardware scan instruction
    (tensor_tensor_scan, op0=mult, op1=add).

Per core: 512K-sample contiguous shard as 128 partition-rows of 4096
samples with a W-sample warmup halo per row.

Engine split per block: scalar: Square, Ln, Exp;  vector: CSEL3, D1SEL3
(fused select ops), scan;  gpsimd: final out = audio * gain;  sync: DMA.
All scalar activations live in one table set (natural_log_exp_and_others);
the per-function set choice is pinned so only one ACT_TABLE_LOAD happens,
and a dummy Exp at program start overlaps that load with the first DMA.
"""

import math
import os

import numpy as np

import sys

if "/opt/trn_rl_repo" not in sys.path:
    sys.path.insert(0, "/opt/trn_rl_repo")

P = 128            # SBUF partitions
C = 4096           # valid samples per partition-row
W = 32             # warmup samples per row
NCORES = 8
SHARD = P * C      # samples per core
T_TOTAL = NCORES * SHARD
NBLK = int(os.environ.get("COMP_NBLK", "6"))
MULT_ENGINE = os.environ.get("COMP_MULT", "vector")  # gpsimd | vector
# Column bounds within a row (fd = W + C). A small first block shortens the
# pipeline fill (first DMA + Square/Ln before the vector engine can start);
# a smaller last block shortens the drain (exp + mult + DMA-out after the
# last scan). Override with COMP_BOUNDS="128,960,..." (must sum to fd).
_BOUNDS_ENV = os.environ.get("COMP_BOUNDS", "")
PAD_VAL = 1e9      # |audio| huge -> gain_reduction = 0 -> matches g=0 initial state


def _register_custom_ops():
    """Fused DVE ops, registered at runtime:
      COMP_CSEL3: out = select((in0 < in1) & (in0 < K2), s0, s1)
      COMP_D1SEL3: out = select((in0 < in1) & (in0 < K2), s0, s1) * max(K2 - in0, 0)
    (in0 = L[t], in1 = L[t-1], K2 via imm2.)
    """
    import concourse.dve_ops as dve_ops
    from concourse.dve_spec import Spec, Src0, Src1, C0, C1, C2, Zero, select, maxx, lower
    from concourse.dve_uop import DveOpSpec

    existing = {o.name: o for o in dve_ops.OPS}
    if "COMP_CSEL3" in existing:
        return existing["COMP_CSEL3"], existing["COMP_D1SEL3"]

    def mk(name, body, reference):
        spec = Spec(body=body, reference=reference)
        row = dve_ops._CUSTOM_DVE_ROW_BASE + len(dve_ops.OPS)
        dve_ops._SUB_OPCODE_FOR_NAME[name] = row
        shas = {}
        for ver in ("v3", "v4"):
            ds = DveOpSpec(name=name, opcode=row, uops=lower(spec, ver=ver),
                           rd1_en=True)
            shas[ver] = ds.sha(ver)
        op = dve_ops.DveOp(name, spec, subdim=False, uops_sha=shas)
        dve_ops.OPS.append(op)
        dve_ops.CUSTOM_DVE_SPECS[name] = spec
        return op

    csel = mk(
        "COMP_CSEL3",
        select((Src0 < Src1) & (Src0 < C2), C0, C1),
        lambda in0, in1, s0, s1, imm2: np.where(
            (in0 < in1) & (in0 < imm2), s0, s1).astype(np.float32),
    )
    d1sel = mk(
        "COMP_D1SEL3",
        select((Src0 < Src1) & (Src0 < C2), C0, C1) * maxx(C2 - Src0, Zero),
        lambda in0, in1, s0, s1, imm2: (np.where(
            (in0 < in1) & (in0 < imm2), s0, s1)
            * np.maximum(imm2 - in0, 0.0)).astype(np.float32),
    )
    return csel, d1sel


def _pin_act_table_set(nc):
    """All four activation functions used here (Square, Ln, Exp) live in the
    `natural_log_exp_and_others` set; the compiler's per-function set choice
    would otherwise thrash between sets (one ACT_TABLE_LOAD is ~2.7us).
    Drop these functions from every other set in the (cached) table map so
    the insertion pass resolves them all to the one set that really does
    contain them."""
    import concourse.mybir as mybir
    from concourse.hw_specs import get_activation_tables

    AF = mybir.ActivationFunctionType
    tabs = get_activation_tables(nc.m.arch)
    mine = {AF.Square, AF.Ln, AF.Exp}
    keep = "natural_log_exp_and_others"
    assert keep in tabs and mine <= tabs[keep]
    for name, fns in tabs.items():
        if name != keep:
            fns -= mine


def _build_program(thr, ratio, att, rel, nblk=NBLK, p=P, c=C, w=W):
    import concourse.bacc as bacc
    import concourse.mybir as mybir
    from concourse.ap import AP
    from concourse.tile import TileContext

    CSEL3, D1SEL3 = _register_custom_ops()

    fp32 = mybir.dt.float32
    AF = mybir.ActivationFunctionType
    ALU = mybir.AluOpType

    shard = p * c
    fd = w + c
    if _BOUNDS_ENV:
        widths = [int(x) for x in _BOUNDS_ENV.split(",")]
    else:
        # taper: small first block (fast fill), small last block (fast drain)
        body = nblk - 2
        first = 96
        last = 320
        bw = (fd - first - last) // body
        widths = [first] + [bw] * body + [last]
        widths[-2] += fd - sum(widths)
    assert sum(widths) == fd and all(x > 0 for x in widths), widths
    nblk = len(widths)
    bounds = [0]
    for x in widths:
        bounds.append(bounds[-1] + x)

    k2 = 1.0 - 1.0 / ratio
    K2 = 2.0 * (thr / 20.0) * math.log(10.0)   # = 2*ln(10^(thr/20))
    exp_scale = -k2 / 2.0

    nc = bacc.Bacc("TRN2", target_bir_lowering=False)
    _pin_act_table_set(nc)

    ain = nc.dram_tensor("a_in", [shard + w], fp32, kind="ExternalInput")
    aout = nc.dram_tensor("a_out", [shard], fp32, kind="ExternalOutput")
    ain_h = ain.ap().tensor
    aout_h = aout.ap().tensor

    with TileContext(nc) as tc:
        with tc.tile_pool(name="pool", bufs=1) as pool:
            aud = pool.tile([p, fd], fp32, tag="aud")
            sq = pool.tile([p, fd], fp32, tag="sq")
            L = pool.tile([p, fd], fp32, tag="L")
            coeff = pool.tile([p, fd], fp32, tag="coeff")
            d1 = pool.tile([p, fd], fp32, tag="d1")
            h = pool.tile([p, fd], fp32, tag="h")
            gain = pool.tile([p, fd], fp32, tag="gain")
            outb = pool.tile([p, fd], fp32, tag="outb")
            scratch = pool.tile([p, 1], fp32, tag="scratch")
            eps = pool.tile([p, 1], fp32, tag="eps")

            # Ln bias constant, Tile-tracked (avoids an all-engine barrier).
            nc.gpsimd.memset(eps[:, 0:1], 1e-10)
            # Dummy activation: hoists the single ACT_TABLE_LOAD to t=0 so
            # it overlaps the first DMA instead of serializing before the
            # first Square.
            zero_ap = nc.const_aps.tensor(0.0, (p, 1))
            nc.scalar.activation(scratch[:, 0:1], zero_ap, AF.Exp)

            def scan(b):
                # scan: h[t] = coeff[t]*h[t-1] + d1[t]
                c0, c1 = bounds[b], bounds[b + 1]
                nc.vector.tensor_tensor_scan(
                    h[:, c0:c1], coeff[:, c0:c1], d1[:, c0:c1],
                    initial=0.0 if b == 0 else h[:, c0 - 1:c0],
                    op0=ALU.mult, op1=ALU.add)

            def back_end(b):
                # gain = exp(-k2/2 * h); out = audio*gain
                c0, c1 = bounds[b], bounds[b + 1]
                v0 = max(c0, w)
                nc.scalar.activation(gain[:, v0:c1], h[:, v0:c1], AF.Exp,
                                     scale=exp_scale)
                if MULT_ENGINE == "gpsimd":
                    nc.gpsimd.tensor_tensor(
                        out=outb[:, v0:c1], in0=aud[:, v0:c1],
                        in1=gain[:, v0:c1], op=ALU.mult)
                else:
                    nc.vector.tensor_tensor(
                        outb[:, v0:c1], aud[:, v0:c1], gain[:, v0:c1],
                        op=ALU.mult)
                dst = AP(aout_h, v0 - w, [[c, p], [1, c1 - v0]])
                nc.sync.dma_start(out=dst, in_=outb[:, v0:c1])

            for b in range(nblk):
                c0, c1 = bounds[b], bounds[b + 1]
                blk = slice(c0, c1)
                # rows: aud[pp, col] = ain[pp*c + col]; rows overlap by w
                src = AP(ain_h, c0, [[c, p], [1, c1 - c0]])
                nc.sync.dma_start(out=aud[:, blk], in_=src)

                # front-end (scalar): sq = a^2 ; L = ln(sq + 1e-10)
                nc.scalar.activation(sq[:, blk], aud[:, blk], AF.Square)
                nc.scalar.activation(L[:, blk], sq[:, blk], AF.Ln,
                                     bias=eps[:, 0:1])

                # coeff/d1 from the shifted-L branch predictor
                if b == 0:
                    # col 0: in0 == in1 -> cond false -> release branch
                    nc.vector._custom_dve(
                        CSEL3, out=coeff[:, 0:1], in0=L[:, 0:1],
                        in1=L[:, 0:1], s0=att, s1=rel, imm2=K2)
                    nc.vector._custom_dve(
                        D1SEL3, out=d1[:, 0:1], in0=L[:, 0:1],
                        in1=L[:, 0:1], s0=1.0 - att, s1=1.0 - rel, imm2=K2)
                    nc.vector._custom_dve(
                        CSEL3, out=coeff[:, 1:c1], in0=L[:, 1:c1],
                        in1=L[:, 0:c1 - 1], s0=att, s1=rel, imm2=K2)
                    nc.vector._custom_dve(
                        D1SEL3, out=d1[:, 1:c1], in0=L[:, 1:c1],
                        in1=L[:, 0:c1 - 1], s0=1.0 - att, s1=1.0 - rel,
                        imm2=K2)
                else:
                    nc.vector._custom_dve(
                        CSEL3, out=coeff[:, blk], in0=L[:, blk],
                        in1=L[:, c0 - 1:c1 - 1], s0=att, s1=rel, imm2=K2)
                    nc.vector._custom_dve(
                        D1SEL3, out=d1[:, blk], in0=L[:, blk],
                        in1=L[:, c0 - 1:c1 - 1], s0=1.0 - att, s1=1.0 - rel,
                        imm2=K2)

                # Software-pipelined emission: the scan for block b-1 goes on
                # the vector queue AFTER block b's selects, so its operands
                # (block b-1's coeff/d1) have drained the DVE pipe by the
                # time it issues -- no read-after-write bubble.  Back-ends
                # lag two blocks for the same reason (exp(b-2) is ready).
                if b > 0:
                    scan(b - 1)
                if b > 1:
                    back_end(b - 2)
            scan(nblk - 1)
            back_end(nblk - 2)
            back_end(nblk - 1)

    if not nc.is_finalized():
        nc.finalize()
    return nc


_CACHE = {}


def _get_program(thr, ratio, att, rel):
    key = (float(thr), float(ratio), float(att), float(rel), NBLK, MULT_ENGINE)
    if key not in _CACHE:
        _CACHE[key] = _build_program(*key[:4], nblk=NBLK)
    return _CACHE[key]


def kernel(audio, threshold, ratio, attack, release):
    from concourse.bass_utils import run_bass_kernel_spmd

    audio = np.asarray(audio, dtype=np.float32)
    assert audio.shape == (T_TOTAL,), audio.shape
    thr = float(np.asarray(threshold))
    rat = float(np.asarray(ratio))
    att = float(np.asarray(attack))
    rel = float(np.asarray(release))

    nc = _get_program(thr, rat, att, rel)

    padded = np.concatenate([np.full(W, PAD_VAL, dtype=np.float32), audio])
    in_maps = [
        {"a_in": padded[cid * SHARD: cid * SHARD + SHARD + W]}
        for cid in range(NCORES)
    ]
    res = run_bass_kernel_spmd(nc, in_maps, list(range(NCORES)))
    out = np.concatenate([res.results[cid]["a_out"] for cid in range(NCORES)])
    return out.astype(np.float32)


# revision 10
# speedup vs baseline: 1.4136x; 1.0718x over previous
"""Trainium2 Bass kernel for a dynamic-range compressor (nn_Compressor).

Reference semantics (fp32):
    audio_db = 20*log10(|audio| + 1e-5)
    gr_db    = max((threshold - audio_db) * (1 - 1/ratio), 0)
    scan:    g[t] = g[t-1] + (1-c)*(gr_db[t] - g[t-1]),  c = attack if gr_db[t] > g[t-1] else release
    out      = audio * 10^(-g/20)

Design notes:
  * The smoother is strongly contracting (c <= 0.1), so g[t-1] ~= x[t-1]:
    the branch is predicted from x[t] vs x[t-2] (a pure shifted view, two
    back so fp16 packed pairs stay 4-byte aligned).  Validated 5.3e-5
    relative output error vs the sequential fp32 reference (gate 2e-2).
  * Work in the ln(a^2) domain: sq = a^2 (Square), L = ln(sq + 1e-10), then
    rt = min(L - K2, 0) with K2 = 2*ln(10^(thr/20)).  rt = -2/gamma * gr_db,
    so the branch compare is rt[t] < rt[t-2], the scan input d1 uses negated
    (1-c) constants, and the leftover scale folds into the final Exp.
    This removes the Abs and Relu scalar passes entirely.
  * coeff/d1 come from two custom fused DVE select ops running in fp16
    packed-pair 2X_1PORT mode (hand-written uop programs, ~2 elem/cycle).
  * Scan: h[t] = coeff[t]*h[t-1] + d1[t] via tensor_tensor_scan (fp16 data,
    fp32 internal state).
  * gain = exp(-k2/2 * h) on the scalar engine; out = audio * gain on the
    vector engine (fp32).  GpSimd is left idle: its SBUF port is shared
    with the DVE and any Pool work blocks 2-port vector instructions.
  * All scalar activations (Square/Ln/Exp) are pinned to the single
    natural_log_exp_and_others table set (one ACT_TABLE_LOAD), and a dummy
    Exp at program start overlaps that load with the first DMA.
  * Per core: 512K-sample contiguous shard as 128 partition-rows of 4096
    samples with a W-sample warmup halo per row.  Column blocks are
    tapered (small first block = fast pipeline fill, small last block =
    fast drain) and the emission order software-pipelines the vector queue
    so scans never stall on the DVE drain of their operand producers.
"""

import math
import os

import numpy as np

import sys

if "/opt/trn_rl_repo" not in sys.path:
    sys.path.insert(0, "/opt/trn_rl_repo")

P = 128            # SBUF partitions
C = 4096           # valid samples per partition-row
W = 32             # warmup samples per row
NCORES = 8
SHARD = P * C      # samples per core
T_TOTAL = NCORES * SHARD
NBLK = int(os.environ.get("COMP_NBLK", "6"))
# Column bounds within a row (fd = W + C).  Overridable for tuning:
# COMP_BOUNDS="96,928,..." (must sum to fd, all even).
_BOUNDS_ENV = os.environ.get("COMP_BOUNDS", "")
PAD_VAL = 1e9      # |audio| huge -> gain_reduction = 0 -> matches g=0 initial state


def _register_custom_ops():
    """Fused DVE select ops with hand-written fp16 2X_1PORT uop programs:
      COMP_CSEL16:  out = select(in0 < in1, s0, s1)
      COMP_D1SEL16: out = select(in0 < in1, s0, s1) * in0
    The 2x program packs fp16 pairs: lo computed in blocks 0-2, hi in
    blocks 3-7, lo rides delay chain 0 to write0_lo, hi leaves from the
    block-7 ALU to write0_hi.  Emitters set perf_max=1 to let the engine's
    mode detection engage the 2x slot (16-bit dtype, step 1, 4B-aligned).
    """
    import concourse.dve_ops as dve_ops
    from concourse.dve_spec import Spec, Src0, Src1, C0, C1, select, lower
    from concourse.dve_uop import (
        DveOpSpec, UopConfig, AluOp, AluInp, DelayInp, InpSel,
        OutPath, OutSel, Trigger, ENABLE,
    )

    existing = {o.name: o for o in dve_ops.OPS}
    if "COMP_CSEL16" in existing:
        return existing["COMP_CSEL16"], existing["COMP_D1SEL16"]

    PD = [AluInp.PREV_DELAY_0, AluInp.PREV_DELAY_1, AluInp.PREV_DELAY_2,
          AluInp.PREV_DELAY_3, AluInp.PREV_DELAY_4, AluInp.PREV_DELAY_5]
    PREV = AluInp.PREV_ALU_OUT

    def base_uop():
        u = UopConfig()
        u.trigger = (Trigger.SRC_TENSOR_DONE, Trigger.NONE, Trigger.NONE)
        u.require_inp0 = ENABLE
        u.require_inp1 = ENABLE
        # lanes: 0=SRC_0(lo) 1=SRC_1(lo) 2=SRC_0_HI 3=SRC_1_HI 4=C0 5=C1
        u.enable_input(InpSel.SRC_0, 0)
        u.enable_input(InpSel.SRC_1, 1)
        u.enable_input(InpSel.SRC_0_HI, 2)
        u.enable_input(InpSel.SRC_1_HI, 3)
        u.enable_input(InpSel.CONST_0, 4)
        u.enable_input(InpSel.CONST_1, 5)
        u.enable_output(OutSel.DELAY_0, OutPath.WR0_LO)   # lo result via chain 0
        u.enable_output(OutSel.ALU_OUT, OutPath.WR0_HI)   # hi result from blk7
        return u

    def csel16_2x():
        u = base_uop()
        b = u.datapath_config
        # blk0: q_hi = (rt_hi < rtp_hi); load chains
        b[0].enable_alu(AluOp.IS_LT, PD[1], PD[2])
        b[0].enable_delay_from_src(DelayInp.PREV_DELAY, 0)    # rtp_lo (lane1)
        b[0].enable_delay_from_src(DelayInp.PREV_ALU_OUT, 1)  # rt_lo (lane0)
        b[0].enable_delay_from_src(DelayInp.PREV_DELAY, 3)    # C0 (lane4)
        b[0].enable_delay_from_src(DelayInp.PREV_DELAY, 4)    # C1 (lane5)
        # blk1: q_lo = (rt_lo < rtp_lo); stash q_hi on chain 2
        b[1].enable_alu(AluOp.IS_LT, PD[1], PD[0])
        b[1].enable_delay_from_src(DelayInp.PREV_ALU_OUT, 2)
        b[1].pass_through_delay(3, 4)
        # blk2: sel_lo = q_lo ? C0 : C1  (SELECT: mux0=false-val, mux1=true-val)
        b[2].enable_alu(AluOp.SELECT, PD[4], PD[3])
        b[2].pass_through_delay(2, 3, 4)
        # blk3: re-materialize q_hi in the ALU; stash sel_lo on chain 0
        b[3].enable_alu(AluOp.BYPASS, PD[2], PD[2])
        b[3].enable_delay_from_src(DelayInp.PREV_ALU_OUT, 0)
        b[3].pass_through_delay(3, 4)
        # blk4: sel_hi
        b[4].enable_alu(AluOp.SELECT, PD[4], PD[3])
        b[4].pass_through_delay(0)
        # blk5..7: carry sel_hi in the ALU, sel_lo on chain 0
        for k in (5, 6, 7):
            b[k].pass_through_alu()
            b[k].pass_through_delay(0)
        u.validate("v3")
        return u

    def d1sel16_2x():
        u = base_uop()
        b = u.datapath_config
        # blk0: q_hi = (rt_hi < rtp_hi)
        b[0].enable_alu(AluOp.IS_LT, PD[1], PD[2])
        b[0].enable_delay_from_src(DelayInp.PREV_DELAY, 0)    # rtp_lo
        b[0].enable_delay_from_src(DelayInp.PREV_DELAY, 1)    # rt_hi (lane2)
        b[0].enable_delay_from_src(DelayInp.PREV_ALU_OUT, 2)  # rt_lo (lane0)
        b[0].enable_delay_from_src(DelayInp.PREV_DELAY, 3)    # C0
        b[0].enable_delay_from_src(DelayInp.PREV_DELAY, 4)    # C1
        # blk1: q_lo = (rt_lo < rtp_lo); stash q_hi on chain 5
        b[1].enable_alu(AluOp.IS_LT, PD[2], PD[0])
        b[1].enable_delay_from_src(DelayInp.PREV_ALU_OUT, 5)
        b[1].pass_through_delay(1, 2, 3, 4)
        # blk2: sel_lo
        b[2].enable_alu(AluOp.SELECT, PD[4], PD[3])
        b[2].pass_through_delay(1, 2, 3, 4, 5)
        # blk3: d1_lo = sel_lo * rt_lo
        b[3].enable_alu(AluOp.MULTIPLY, PREV, PD[2])
        b[3].pass_through_delay(1, 3, 4, 5)
        # blk4: re-materialize q_hi; stash d1_lo on chain 0
        b[4].enable_alu(AluOp.BYPASS, PD[5], PD[5])
        b[4].enable_delay_from_src(DelayInp.PREV_ALU_OUT, 0)
        b[4].pass_through_delay(1, 3, 4)
        # blk5: sel_hi
        b[5].enable_alu(AluOp.SELECT, PD[4], PD[3])
        b[5].pass_through_delay(0, 1)
        # blk6: d1_hi = sel_hi * rt_hi
        b[6].enable_alu(AluOp.MULTIPLY, PREV, PD[1])
        b[6].pass_through_delay(0)
        # blk7: carry
        b[7].pass_through_alu()
        b[7].pass_through_delay(0)
        u.validate("v3")
        return u

    def mk(name, body, reference, uop2x):
        spec = Spec(body=body, reference=reference)
        row = dve_ops._CUSTOM_DVE_ROW_BASE + len(dve_ops.OPS)
        dve_ops._SUB_OPCODE_FOR_NAME[name] = row
        shas = {}
        compiled = {}
        for ver in ("v3", "v4"):
            ds = DveOpSpec(name=name, opcode=row, uops=lower(spec, ver=ver),
                           rd1_en=True,
                           uops_2x=([uop2x] if ver == "v3" else None),
                           perf_max=(1 if ver == "v3" else 0))
            shas[ver] = ds.sha(ver)
            compiled[ver] = ds
        op = dve_ops.DveOp(name, spec, subdim=False, uops_sha=shas,
                           perf_en={"v3": True})
        dve_ops.OPS.append(op)
        dve_ops.CUSTOM_DVE_SPECS[name] = spec
        # Pre-seed the compile cache with the spec carrying the 2x program
        # (DveOp.compile would re-lower from the DSL and drop it).
        for ver in ("v3", "v4"):
            dve_ops._COMPILE_CACHE[(name, ver)] = compiled[ver]
        return op

    csel = mk(
        "COMP_CSEL16", select(Src0 < Src1, C0, C1),
        lambda in0, in1, s0, s1, imm2: np.where(in0 < in1, s0, s1).astype(np.float32),
        csel16_2x(),
    )
    d1sel = mk(
        "COMP_D1SEL16", select(Src0 < Src1, C0, C1) * Src0,
        lambda in0, in1, s0, s1, imm2: (np.where(in0 < in1, s0, s1) * in0).astype(np.float32),
        d1sel16_2x(),
    )
    return csel, d1sel


def _pin_act_table_set(nc):
    """Square/Ln/Exp all live in `natural_log_exp_and_others`; the compiler's
    per-function set choice would otherwise thrash between sets (each
    ACT_TABLE_LOAD is ~2.7us).  Drop these functions from every other set in
    the (cached) table map so the insertion pass resolves them all to the one
    set that really does contain them."""
    import concourse.mybir as mybir
    from concourse.hw_specs import get_activation_tables

    AF = mybir.ActivationFunctionType
    tabs = get_activation_tables(nc.m.arch)
    mine = {AF.Square, AF.Ln, AF.Exp}
    keep = "natural_log_exp_and_others"
    assert keep in tabs and mine <= tabs[keep]
    for name, fns in tabs.items():
        if name != keep:
            fns -= mine


def _build_program(thr, ratio, att, rel, nblk=NBLK, p=P, c=C, w=W):
    import concourse.bacc as bacc
    import concourse.mybir as mybir
    from concourse.ap import AP
    from concourse.tile import TileContext

    CSEL16, D1SEL16 = _register_custom_ops()

    fp32 = mybir.dt.float32
    fp16 = mybir.dt.float16
    AF = mybir.ActivationFunctionType
    ALU = mybir.AluOpType

    shard = p * c
    fd = w + c
    if _BOUNDS_ENV:
        widths = [int(x) for x in _BOUNDS_ENV.split(",")]
    else:
        # taper: small first block (fast fill), small last block (fast drain)
        body = nblk - 2
        first = 96
        last = 320
        bw = (fd - first - last) // body // 2 * 2
        widths = [first] + [bw] * body + [last]
        widths[-2] += fd - sum(widths)
    assert sum(widths) == fd and all(x > 0 and x % 2 == 0 for x in widths), widths
    nblk = len(widths)
    bounds = [0]
    for x in widths:
        bounds.append(bounds[-1] + x)

    k2 = 1.0 - 1.0 / ratio
    K2 = 2.0 * (thr / 20.0) * math.log(10.0)   # = 2*ln(10^(thr/20))
    exp_scale = -k2 / 2.0

    nc = bacc.Bacc("TRN2", target_bir_lowering=False)
    _pin_act_table_set(nc)

    ain = nc.dram_tensor("a_in", [shard + w], fp32, kind="ExternalInput")
    aout = nc.dram_tensor("a_out", [shard], fp32, kind="ExternalOutput")
    ain_h = ain.ap().tensor
    aout_h = aout.ap().tensor

    with TileContext(nc) as tc:
        with tc.tile_pool(name="pool", bufs=1) as pool:
            aud = pool.tile([p, fd], fp32, tag="aud")
            sq = pool.tile([p, fd], fp32, tag="sq")
            L = pool.tile([p, fd], fp16, tag="L")
            rt = pool.tile([p, fd], fp16, tag="rt")
            coeff = pool.tile([p, fd], fp16, tag="coeff")
            d1 = pool.tile([p, fd], fp16, tag="d1")
            h = pool.tile([p, fd], fp16, tag="h")
            gain = pool.tile([p, fd], fp32, tag="gain")
            outb = pool.tile([p, fd], fp32, tag="outb")
            scratch = pool.tile([p, 1], fp32, tag="scratch")
            eps = pool.tile([p, 1], fp32, tag="eps")

            # Ln bias constant, Tile-tracked (avoids an all-engine barrier).
            nc.gpsimd.memset(eps[:, 0:1], 1e-10)
            # Dummy activation: hoists the single ACT_TABLE_LOAD to t=0 so it
            # overlaps the first DMA instead of serializing before Square.
            zero_ap = nc.const_aps.tensor(0.0, (p, 1))
            nc.scalar.activation(scratch[:, 0:1], zero_ap, AF.Exp)

            def selects(b):
                c0, c1 = bounds[b], bounds[b + 1]
                blk = slice(c0, c1)
                # rt = min(L - K2, 0)  (single-src fp16 -> 4x tensor_scalar)
                nc.vector.tensor_scalar(
                    out=rt[:, blk], in0=L[:, blk], scalar1=K2, scalar2=0.0,
                    op0=ALU.subtract, op1=ALU.min)
                if b == 0:
                    # cols 0-1: in0 == in1 -> cond false -> release branch
                    i0 = nc.vector._custom_dve(
                        CSEL16, out=coeff[:, 0:2], in0=rt[:, 0:2],
                        in1=rt[:, 0:2], s0=att, s1=rel)
                    i1 = nc.vector._custom_dve(
                        D1SEL16, out=d1[:, 0:2], in0=rt[:, 0:2],
                        in1=rt[:, 0:2], s0=-(1.0 - att), s1=-(1.0 - rel))
                    i2 = nc.vector._custom_dve(
                        CSEL16, out=coeff[:, 2:c1], in0=rt[:, 2:c1],
                        in1=rt[:, 0:c1 - 2], s0=att, s1=rel)
                    i3 = nc.vector._custom_dve(
                        D1SEL16, out=d1[:, 2:c1], in0=rt[:, 2:c1],
                        in1=rt[:, 0:c1 - 2], s0=-(1.0 - att), s1=-(1.0 - rel))
                    insts = (i0, i1, i2, i3)
                else:
                    i2 = nc.vector._custom_dve(
                        CSEL16, out=coeff[:, blk], in0=rt[:, blk],
                        in1=rt[:, c0 - 2:c1 - 2], s0=att, s1=rel)
                    i3 = nc.vector._custom_dve(
                        D1SEL16, out=d1[:, blk], in0=rt[:, blk],
                        in1=rt[:, c0 - 2:c1 - 2], s0=-(1.0 - att),
                        s1=-(1.0 - rel))
                    insts = (i2, i3)
                for bi in insts:
                    bi.ins.perf_max = 1

            def scan(b):
                # h[t] = coeff[t]*h[t-1] + d1[t]  (fp32 internal state)
                c0, c1 = bounds[b], bounds[b + 1]
                nc.vector.tensor_tensor_scan(
                    h[:, c0:c1], coeff[:, c0:c1], d1[:, c0:c1],
                    initial=0.0 if b == 0 else h[:, c0 - 1:c0],
                    op0=ALU.mult, op1=ALU.add)

            def back_end(b):
                # gain = exp(-k2/2 * h); out = audio*gain
                c0, c1 = bounds[b], bounds[b + 1]
                v0 = max(c0, w)
                nc.scalar.activation(gain[:, v0:c1], h[:, v0:c1], AF.Exp,
                                     scale=exp_scale)
                nc.vector.tensor_tensor(
                    outb[:, v0:c1], aud[:, v0:c1], gain[:, v0:c1],
                    op=ALU.mult)
                dst = AP(aout_h, v0 - w, [[c, p], [1, c1 - v0]])
                nc.sync.dma_start(out=dst, in_=outb[:, v0:c1])

            for b in range(nblk):
                c0, c1 = bounds[b], bounds[b + 1]
                blk = slice(c0, c1)
                # rows: aud[pp, col] = ain[pp*c + col]; rows overlap by w
                src = AP(ain_h, c0, [[c, p], [1, c1 - c0]])
                nc.sync.dma_start(out=aud[:, blk], in_=src)

                # front-end (scalar): sq = a^2 ; L = ln(sq + 1e-10)
                nc.scalar.activation(sq[:, blk], aud[:, blk], AF.Square)
                nc.scalar.activation(L[:, blk], sq[:, blk], AF.Ln,
                                     bias=eps[:, 0:1])

                selects(b)
                # Software-pipelined emission: block b-1's scan goes on the
                # vector queue after block b's selects so its operands have
                # drained the DVE pipe when it issues; back-ends lag two.
                if b > 0:
                    scan(b - 1)
                if b > 1:
                    back_end(b - 2)
            scan(nblk - 1)
            back_end(nblk - 2)
            back_end(nblk - 1)

    if not nc.is_finalized():
        nc.finalize()
    return nc


_CACHE = {}


def _get_program(thr, ratio, att, rel):
    key = (float(thr), float(ratio), float(att), float(rel), NBLK, _BOUNDS_ENV)
    if key not in _CACHE:
        _CACHE[key] = _build_program(*key[:4], nblk=NBLK)
    return _CACHE[key]


def kernel(audio, threshold, ratio, attack, release):
    from concourse.bass_utils import run_bass_kernel_spmd

    audio = np.asarray(audio, dtype=np.float32)
    assert audio.shape == (T_TOTAL,), audio.shape
    thr = float(np.asarray(threshold))
    rat = float(np.asarray(ratio))
    att = float(np.asarray(attack))
    rel = float(np.asarray(release))

    nc = _get_program(thr, rat, att, rel)

    padded = np.concatenate([np.full(W, PAD_VAL, dtype=np.float32), audio])
    in_maps = [
        {"a_in": padded[cid * SHARD: cid * SHARD + SHARD + W]}
        for cid in range(NCORES)
    ]
    res = run_bass_kernel_spmd(nc, in_maps, list(range(NCORES)))
    out = np.concatenate([res.results[cid]["a_out"] for cid in range(NCORES)])
    return out.astype(np.float32)


# revision 12
# speedup vs baseline: 1.4562x; 1.0301x over previous
"""Trainium2 Bass kernel for a dynamic-range compressor (nn_Compressor).

Reference semantics (fp32):
    audio_db = 20*log10(|audio| + 1e-5)
    gr_db    = max((threshold - audio_db) * (1 - 1/ratio), 0)
    scan:    g[t] = g[t-1] + (1-c)*(gr_db[t] - g[t-1]),  c = attack if gr_db[t] > g[t-1] else release
    out      = audio * 10^(-g/20)

Design notes:
  * The smoother is strongly contracting (c <= 0.1), so g[t-1] ~= x[t-1]:
    the branch is predicted from x[t] vs x[t-2] (a pure shifted view, two
    back so fp16 packed pairs stay 4-byte aligned).  Validated 5.3e-5
    relative output error vs the sequential fp32 reference (gate 2e-2).
  * Work in the ln(a^2) domain: sq = a^2 (Square), L = ln(sq + 1e-10), then
    rt = min(L - K2, 0) with K2 = 2*ln(10^(thr/20)).  rt = -2/gamma * gr_db,
    so the branch compare is rt[t] < rt[t-2], the scan input d1 uses negated
    (1-c) constants, and the leftover scale folds into the final Exp.
    This removes the Abs and Relu scalar passes entirely.
  * coeff/d1 come from two custom fused DVE select ops running in fp16
    packed-pair 2X_1PORT mode (hand-written uop programs, ~2 elem/cycle).
  * Scan: h[t] = coeff[t]*h[t-1] + d1[t] via tensor_tensor_scan (fp16 data,
    fp32 internal state).
  * gain = exp(-k2/2 * h) on the scalar engine; out = audio * gain on the
    vector engine (fp32).  GpSimd is left idle: its SBUF port is shared
    with the DVE and any Pool work blocks 2-port vector instructions.
  * All scalar activations (Square/Ln/Exp) are pinned to the single
    natural_log_exp_and_others table set (one ACT_TABLE_LOAD), and a dummy
    Exp at program start overlaps that load with the first DMA.
  * Per core: 512K-sample contiguous shard as 128 partition-rows of 4096
    samples with a W-sample warmup halo per row.  Column blocks are
    tapered (small first block = fast pipeline fill, small last block =
    fast drain) and the emission order software-pipelines the vector queue
    so scans never stall on the DVE drain of their operand producers.
"""

import math
import os

import numpy as np

import sys

if "/opt/trn_rl_repo" not in sys.path:
    sys.path.insert(0, "/opt/trn_rl_repo")

P = 128            # SBUF partitions
C = 4096           # valid samples per partition-row
W = 32             # warmup samples per row
NCORES = 8
SHARD = P * C      # samples per core
T_TOTAL = NCORES * SHARD
NBLK = int(os.environ.get("COMP_NBLK", "6"))
# Column bounds within a row (fd = W + C).  Overridable for tuning:
# COMP_BOUNDS="96,928,..." (must sum to fd, all even).
_BOUNDS_ENV = os.environ.get("COMP_BOUNDS", "")
PAD_VAL = 1e9      # |audio| huge -> gain_reduction = 0 -> matches g=0 initial state


def _register_custom_ops():
    """Fused DVE select ops with hand-written fp16 2X_1PORT uop programs:
      COMP_CSEL16:  out = select(in0 < in1, s0, s1)
      COMP_D1SEL16: out = select(in0 < in1, s0, s1) * in0
    The 2x program packs fp16 pairs: lo computed in blocks 0-2, hi in
    blocks 3-7, lo rides delay chain 0 to write0_lo, hi leaves from the
    block-7 ALU to write0_hi.  Emitters set perf_max=1 to let the engine's
    mode detection engage the 2x slot (16-bit dtype, step 1, 4B-aligned).
    """
    import concourse.dve_ops as dve_ops
    from concourse.dve_spec import Spec, Src0, Src1, C0, C1, select, lower
    from concourse.dve_uop import (
        DveOpSpec, UopConfig, AluOp, AluInp, DelayInp, InpSel,
        OutPath, OutSel, Trigger, ENABLE,
    )

    existing = {o.name: o for o in dve_ops.OPS}
    if "COMP_CSEL16" in existing:
        return existing["COMP_CSEL16"], existing["COMP_D1SEL16"]

    PD = [AluInp.PREV_DELAY_0, AluInp.PREV_DELAY_1, AluInp.PREV_DELAY_2,
          AluInp.PREV_DELAY_3, AluInp.PREV_DELAY_4, AluInp.PREV_DELAY_5]
    PREV = AluInp.PREV_ALU_OUT

    def base_uop():
        u = UopConfig()
        u.trigger = (Trigger.SRC_TENSOR_DONE, Trigger.NONE, Trigger.NONE)
        u.require_inp0 = ENABLE
        u.require_inp1 = ENABLE
        # lanes: 0=SRC_0(lo) 1=SRC_1(lo) 2=SRC_0_HI 3=SRC_1_HI 4=C0 5=C1
        u.enable_input(InpSel.SRC_0, 0)
        u.enable_input(InpSel.SRC_1, 1)
        u.enable_input(InpSel.SRC_0_HI, 2)
        u.enable_input(InpSel.SRC_1_HI, 3)
        u.enable_input(InpSel.CONST_0, 4)
        u.enable_input(InpSel.CONST_1, 5)
        u.enable_output(OutSel.DELAY_0, OutPath.WR0_LO)   # lo result via chain 0
        u.enable_output(OutSel.ALU_OUT, OutPath.WR0_HI)   # hi result from blk7
        return u

    def csel16_2x():
        u = base_uop()
        b = u.datapath_config
        # blk0: q_hi = (rt_hi < rtp_hi); load chains
        b[0].enable_alu(AluOp.IS_LT, PD[1], PD[2])
        b[0].enable_delay_from_src(DelayInp.PREV_DELAY, 0)    # rtp_lo (lane1)
        b[0].enable_delay_from_src(DelayInp.PREV_ALU_OUT, 1)  # rt_lo (lane0)
        b[0].enable_delay_from_src(DelayInp.PREV_DELAY, 3)    # C0 (lane4)
        b[0].enable_delay_from_src(DelayInp.PREV_DELAY, 4)    # C1 (lane5)
        # blk1: q_lo = (rt_lo < rtp_lo); stash q_hi on chain 2
        b[1].enable_alu(AluOp.IS_LT, PD[1], PD[0])
        b[1].enable_delay_from_src(DelayInp.PREV_ALU_OUT, 2)
        b[1].pass_through_delay(3, 4)
        # blk2: sel_lo = q_lo ? C0 : C1  (SELECT: mux0=false-val, mux1=true-val)
        b[2].enable_alu(AluOp.SELECT, PD[4], PD[3])
        b[2].pass_through_delay(2, 3, 4)
        # blk3: re-materialize q_hi in the ALU; stash sel_lo on chain 0
        b[3].enable_alu(AluOp.BYPASS, PD[2], PD[2])
        b[3].enable_delay_from_src(DelayInp.PREV_ALU_OUT, 0)
        b[3].pass_through_delay(3, 4)
        # blk4: sel_hi
        b[4].enable_alu(AluOp.SELECT, PD[4], PD[3])
        b[4].pass_through_delay(0)
        # blk5..7: carry sel_hi in the ALU, sel_lo on chain 0
        for k in (5, 6, 7):
            b[k].pass_through_alu()
            b[k].pass_through_delay(0)
        u.validate("v3")
        return u

    def d1sel16_2x():
        u = base_uop()
        b = u.datapath_config
        # blk0: q_hi = (rt_hi < rtp_hi)
        b[0].enable_alu(AluOp.IS_LT, PD[1], PD[2])
        b[0].enable_delay_from_src(DelayInp.PREV_DELAY, 0)    # rtp_lo
        b[0].enable_delay_from_src(DelayInp.PREV_DELAY, 1)    # rt_hi (lane2)
        b[0].enable_delay_from_src(DelayInp.PREV_ALU_OUT, 2)  # rt_lo (lane0)
        b[0].enable_delay_from_src(DelayInp.PREV_DELAY, 3)    # C0
        b[0].enable_delay_from_src(DelayInp.PREV_DELAY, 4)    # C1
        # blk1: q_lo = (rt_lo < rtp_lo); stash q_hi on chain 5
        b[1].enable_alu(AluOp.IS_LT, PD[2], PD[0])
        b[1].enable_delay_from_src(DelayInp.PREV_ALU_OUT, 5)
        b[1].pass_through_delay(1, 2, 3, 4)
        # blk2: sel_lo
        b[2].enable_alu(AluOp.SELECT, PD[4], PD[3])
        b[2].pass_through_delay(1, 2, 3, 4, 5)
        # blk3: d1_lo = sel_lo * rt_lo
        b[3].enable_alu(AluOp.MULTIPLY, PREV, PD[2])
        b[3].pass_through_delay(1, 3, 4, 5)
        # blk4: re-materialize q_hi; stash d1_lo on chain 0
        b[4].enable_alu(AluOp.BYPASS, PD[5], PD[5])
        b[4].enable_delay_from_src(DelayInp.PREV_ALU_OUT, 0)
        b[4].pass_through_delay(1, 3, 4)
        # blk5: sel_hi
        b[5].enable_alu(AluOp.SELECT, PD[4], PD[3])
        b[5].pass_through_delay(0, 1)
        # blk6: d1_hi = sel_hi * rt_hi
        b[6].enable_alu(AluOp.MULTIPLY, PREV, PD[1])
        b[6].pass_through_delay(0)
        # blk7: carry
        b[7].pass_through_alu()
        b[7].pass_through_delay(0)
        u.validate("v3")
        return u

    def mk(name, body, reference, uop2x):
        spec = Spec(body=body, reference=reference)
        row = dve_ops._CUSTOM_DVE_ROW_BASE + len(dve_ops.OPS)
        dve_ops._SUB_OPCODE_FOR_NAME[name] = row
        shas = {}
        compiled = {}
        for ver in ("v3", "v4"):
            ds = DveOpSpec(name=name, opcode=row, uops=lower(spec, ver=ver),
                           rd1_en=True,
                           uops_2x=([uop2x] if ver == "v3" else None),
                           perf_max=(1 if ver == "v3" else 0))
            shas[ver] = ds.sha(ver)
            compiled[ver] = ds
        op = dve_ops.DveOp(name, spec, subdim=False, uops_sha=shas,
                           perf_en={"v3": True})
        dve_ops.OPS.append(op)
        dve_ops.CUSTOM_DVE_SPECS[name] = spec
        # Pre-seed the compile cache with the spec carrying the 2x program
        # (DveOp.compile would re-lower from the DSL and drop it).
        for ver in ("v3", "v4"):
            dve_ops._COMPILE_CACHE[(name, ver)] = compiled[ver]
        return op

    csel = mk(
        "COMP_CSEL16", select(Src0 < Src1, C0, C1),
        lambda in0, in1, s0, s1, imm2: np.where(in0 < in1, s0, s1).astype(np.float32),
        csel16_2x(),
    )
    d1sel = mk(
        "COMP_D1SEL16", select(Src0 < Src1, C0, C1) * Src0,
        lambda in0, in1, s0, s1, imm2: (np.where(in0 < in1, s0, s1) * in0).astype(np.float32),
        d1sel16_2x(),
    )
    return csel, d1sel


def _pin_act_table_set(nc):
    """Square/Ln/Exp all live in `natural_log_exp_and_others`; the compiler's
    per-function set choice would otherwise thrash between sets (each
    ACT_TABLE_LOAD is ~2.7us).  Drop these functions from every other set in
    the (cached) table map so the insertion pass resolves them all to the one
    set that really does contain them."""
    import concourse.mybir as mybir
    from concourse.hw_specs import get_activation_tables

    AF = mybir.ActivationFunctionType
    tabs = get_activation_tables(nc.m.arch)
    mine = {AF.Square, AF.Ln, AF.Exp}
    keep = "natural_log_exp_and_others"
    assert keep in tabs and mine <= tabs[keep]
    for name, fns in tabs.items():
        if name != keep:
            fns -= mine


def _build_program(thr, ratio, att, rel, nblk=NBLK, p=P, c=C, w=W):
    import concourse.bacc as bacc
    import concourse.mybir as mybir
    from concourse.ap import AP
    from concourse.tile import TileContext

    CSEL16, D1SEL16 = _register_custom_ops()

    fp32 = mybir.dt.float32
    fp16 = mybir.dt.float16
    AF = mybir.ActivationFunctionType
    ALU = mybir.AluOpType

    shard = p * c
    fd = w + c
    if _BOUNDS_ENV:
        widths = [int(x) for x in _BOUNDS_ENV.split(",")]
    else:
        # taper: small early blocks (fast pipeline fill while the first DMA
        # and Square/Ln chain land), small last block (fast drain)
        widths = [96, 256, 640, 1024, 1024, 832, 256]
    assert sum(widths) == fd and all(x > 0 and x % 2 == 0 for x in widths), widths
    nblk = len(widths)
    bounds = [0]
    for x in widths:
        bounds.append(bounds[-1] + x)

    k2 = 1.0 - 1.0 / ratio
    K2 = 2.0 * (thr / 20.0) * math.log(10.0)   # = 2*ln(10^(thr/20))
    exp_scale = -k2 / 2.0

    nc = bacc.Bacc("TRN2", target_bir_lowering=False)
    _pin_act_table_set(nc)

    ain = nc.dram_tensor("a_in", [shard + w], fp32, kind="ExternalInput")
    aout = nc.dram_tensor("a_out", [shard], fp32, kind="ExternalOutput")
    ain_h = ain.ap().tensor
    aout_h = aout.ap().tensor

    with TileContext(nc) as tc:
        with tc.tile_pool(name="pool", bufs=1) as pool:
            aud = pool.tile([p, fd], fp32, tag="aud")
            sq = pool.tile([p, fd], fp32, tag="sq")
            L = pool.tile([p, fd], fp16, tag="L")
            rt = pool.tile([p, fd], fp16, tag="rt")
            coeff = pool.tile([p, fd], fp16, tag="coeff")
            d1 = pool.tile([p, fd], fp16, tag="d1")
            h = pool.tile([p, fd], fp16, tag="h")
            gain = pool.tile([p, fd], fp32, tag="gain")
            outb = pool.tile([p, fd], fp32, tag="outb")
            scratch = pool.tile([p, 1], fp32, tag="scratch")
            eps = pool.tile([p, 1], fp32, tag="eps")

            # Ln bias constant, Tile-tracked (avoids an all-engine barrier).
            nc.gpsimd.memset(eps[:, 0:1], 1e-10)
            # Dummy activation: hoists the single ACT_TABLE_LOAD to t=0 so it
            # overlaps the first DMA instead of serializing before Square.
            zero_ap = nc.const_aps.tensor(0.0, (p, 1))
            nc.scalar.activation(scratch[:, 0:1], zero_ap, AF.Exp)

            def selects(b):
                c0, c1 = bounds[b], bounds[b + 1]
                blk = slice(c0, c1)
                # rt = min(L - K2, 0)  (single-src fp16 -> 4x tensor_scalar)
                nc.vector.tensor_scalar(
                    out=rt[:, blk], in0=L[:, blk], scalar1=K2, scalar2=0.0,
                    op0=ALU.subtract, op1=ALU.min)
                if b == 0:
                    # cols 0-1: in0 == in1 -> cond false -> release branch
                    i0 = nc.vector._custom_dve(
                        CSEL16, out=coeff[:, 0:2], in0=rt[:, 0:2],
                        in1=rt[:, 0:2], s0=att, s1=rel)
                    i1 = nc.vector._custom_dve(
                        D1SEL16, out=d1[:, 0:2], in0=rt[:, 0:2],
                        in1=rt[:, 0:2], s0=-(1.0 - att), s1=-(1.0 - rel))
                    i2 = nc.vector._custom_dve(
                        CSEL16, out=coeff[:, 2:c1], in0=rt[:, 2:c1],
                        in1=rt[:, 0:c1 - 2], s0=att, s1=rel)
                    i3 = nc.vector._custom_dve(
                        D1SEL16, out=d1[:, 2:c1], in0=rt[:, 2:c1],
                        in1=rt[:, 0:c1 - 2], s0=-(1.0 - att), s1=-(1.0 - rel))
                    insts = (i0, i1, i2, i3)
                else:
                    i2 = nc.vector._custom_dve(
                        CSEL16, out=coeff[:, blk], in0=rt[:, blk],
                        in1=rt[:, c0 - 2:c1 - 2], s0=att, s1=rel)
                    i3 = nc.vector._custom_dve(
                        D1SEL16, out=d1[:, blk], in0=rt[:, blk],
                        in1=rt[:, c0 - 2:c1 - 2], s0=-(1.0 - att),
                        s1=-(1.0 - rel))
                    insts = (i2, i3)
                for bi in insts:
                    bi.ins.perf_max = 1

            def scan(b):
                # h[t] = coeff[t]*h[t-1] + d1[t]  (fp32 internal state)
                c0, c1 = bounds[b], bounds[b + 1]
                nc.vector.tensor_tensor_scan(
                    h[:, c0:c1], coeff[:, c0:c1], d1[:, c0:c1],
                    initial=0.0 if b == 0 else h[:, c0 - 1:c0],
                    op0=ALU.mult, op1=ALU.add)

            def back_end(b):
                # gain = exp(-k2/2 * h); out = audio*gain
                c0, c1 = bounds[b], bounds[b + 1]
                v0 = max(c0, w)
                nc.scalar.activation(gain[:, v0:c1], h[:, v0:c1], AF.Exp,
                                     scale=exp_scale)
                nc.vector.tensor_tensor(
                    outb[:, v0:c1], aud[:, v0:c1], gain[:, v0:c1],
                    op=ALU.mult)
                dst = AP(aout_h, v0 - w, [[c, p], [1, c1 - v0]])
                nc.sync.dma_start(out=dst, in_=outb[:, v0:c1])

            for b in range(nblk):
                c0, c1 = bounds[b], bounds[b + 1]
                blk = slice(c0, c1)
                # rows: aud[pp, col] = ain[pp*c + col]; rows overlap by w
                src = AP(ain_h, c0, [[c, p], [1, c1 - c0]])
                nc.sync.dma_start(out=aud[:, blk], in_=src)

                # front-end (scalar): sq = a^2 ; L = ln(sq + 1e-10)
                nc.scalar.activation(sq[:, blk], aud[:, blk], AF.Square)
                nc.scalar.activation(L[:, blk], sq[:, blk], AF.Ln,
                                     bias=eps[:, 0:1])

                selects(b)
                # Software-pipelined emission: block b-1's scan goes on the
                # vector queue after block b's selects so its operands have
                # drained the DVE pipe when it issues; back-ends lag two and
                # go ahead of the scan so exp/mult/DMA-out drain earliest.
                if b > 1:
                    back_end(b - 2)
                if b > 0:
                    scan(b - 1)
            back_end(nblk - 2)
            scan(nblk - 1)
            back_end(nblk - 1)

    if not nc.is_finalized():
        nc.finalize()
    return nc


_CACHE = {}


def _get_program(thr, ratio, att, rel):
    key = (float(thr), float(ratio), float(att), float(rel), NBLK, _BOUNDS_ENV)
    if key not in _CACHE:
        _CACHE[key] = _build_program(*key[:4], nblk=NBLK)
    return _CACHE[key]


def kernel(audio, threshold, ratio, attack, release):
    from concourse.bass_utils import run_bass_kernel_spmd

    audio = np.asarray(audio, dtype=np.float32)
    assert audio.shape == (T_TOTAL,), audio.shape
    thr = float(np.asarray(threshold))
    rat = float(np.asarray(ratio))
    att = float(np.asarray(attack))
    rel = float(np.asarray(release))

    nc = _get_program(thr, rat, att, rel)

    padded = np.concatenate([np.full(W, PAD_VAL, dtype=np.float32), audio])
    in_maps = [
        {"a_in": padded[cid * SHARD: cid * SHARD + SHARD + W]}
        for cid in range(NCORES)
    ]
    res = run_bass_kernel_spmd(nc, in_maps, list(range(NCORES)))
    out = np.concatenate([res.results[cid]["a_out"] for cid in range(NCORES)])
    return out.astype(np.float32)
